# revision 2
# baseline (speedup 1.0000x reference)
"""Trainium2 Bass kernel for nn_GraphPooler (segment mean/max pooling + MLP).

Computation (reference):
    mean/max-pool self_feats [2e6, 128] over 10000 contiguous 200-node graphs,
    concat -> [10000, 256], 3-layer MLP -> sigmoid -> [10000, 1].

Strategy (8 NeuronCores, data-parallel over graphs):
  - Each core handles 1280 graphs (256000 node rows, ~131 MB fp32 read).
    Cores 0-6 start at graph 1250*c; core 7 starts at 8720 so its 1280-graph
    window ends exactly at graph 10000 (overlapping outputs are discarded).
  - Per 16-graph "period" (3200 nodes), a SWDGE DMA loads a [128, 25*128]
    chunk, casting fp32->fp16 inline.  Node mapping node = 25*p + r puts all
    25 nodes of partition p inside graph p//8 (200 = 8*25), with 12.8 KB
    contiguous DRAM reads per partition.
  - Max path: a 6-op fp16 tensor_tensor max TREE on DVE folds the 25
    node-slices down to a [128, 128] per-partition partial (packed fp16
    SBUF operands run the DVE at 2 elem/cycle, ~2x faster than a single
    TensorReduce pass, and no PSUM round-trip for the bulk data).
    Partials for 4 periods collect in a [128, 512] fp16 tile; one PE
    transpose per period moves them to PSUM, and one small DVE reduce per
    4 periods (axis m=8 partitions/graph) yields maxT columns.
  - Mean path: per period, 25 K-accumulating matmuls against a 0/1 block
    indicator produce exact fp32 per-graph sums [128d, 16g] in PSUM
    (the /200 scale is folded into W0's mean-half on the host); the
    Activation engine drains them per 4 periods into meanT.
  - MLP: W0 as lhsT in two 128-row K-halves (mean-half @ meanT + max-half
    @ maxT accumulated in PSUM), W1 likewise over h1's halves, W2 -> [1, G],
    sigmoid, DMA out.

The harness calls kernel(**inputs) with the full unsharded inputs and
expects the full [10000, 1] fp32 output.
"""

import numpy as np

import concourse.bacc as bacc
import concourse.tile as tile
from concourse import mybir
from concourse.bass_utils import run_bass_kernel_spmd

F32 = mybir.dt.float32
F16 = mybir.dt.float16
AF = mybir.ActivationFunctionType
AX = mybir.AxisListType
ALU = mybir.AluOpType

NCORES = 8
N_GRAPHS = 10000
NPG = 200          # nodes per graph
D = 128
GP = 16            # graphs per period
JCOLS = 25         # node columns per period chunk (200 = 8 partitions * 25)
PERIOD_NODES = GP * NPG  # 3200
G_CORE = 1280      # graphs computed per core (64-aligned)
N_PERIODS = G_CORE // GP  # 80
SP = 4             # periods per superperiod
N_SP = N_PERIODS // SP  # 20
CORE_ROWS = G_CORE * NPG  # 256000

# graph offset of each core's 1280-graph window; core 7 is pulled back so the
# window ends at graph 10000.  kept output = local graphs [KEEP, KEEP+1250).
CORE_G0 = [1250 * c for c in range(7)] + [N_GRAPHS - G_CORE]
PER_CORE_OUT = N_GRAPHS // NCORES  # 1250


def build_program(reps: int = 1, debug_pooled: bool = False):
    """Build the SPMD Bass program (identical on all 8 cores).

    reps > 1 wraps the whole compute in a hardware For-loop so test harnesses
    can measure steady-state device time via wall-clock deltas.
    """
    nc = bacc.Bacc("TRN2", target_bir_lowering=False, num_devices=NCORES)
    pooled_dbg = (
        nc.dram_tensor("pooled_dbg", [128, 2 * G_CORE], F32, kind="ExternalOutput")
        if debug_pooled
        else None
    )

    feats = nc.dram_tensor("feats", [CORE_ROWS, D], F32, kind="ExternalInput")
    ident = nc.dram_tensor("ident", [128, 128], F16, kind="ExternalInput")
    ind = nc.dram_tensor("ind", [128, GP], F16, kind="ExternalInput")
    w0m = nc.dram_tensor("w0m", [128, 256], F32, kind="ExternalInput")
    w0x = nc.dram_tensor("w0x", [128, 256], F32, kind="ExternalInput")
    w1 = nc.dram_tensor("w1", [256, 128], F32, kind="ExternalInput")
    w2 = nc.dram_tensor("w2", [128, 1], F32, kind="ExternalInput")
    b0 = nc.dram_tensor("b0", [256], F32, kind="ExternalInput")
    b1 = nc.dram_tensor("b1", [128], F32, kind="ExternalInput")
    b2 = nc.dram_tensor("b2", [1], F32, kind="ExternalInput")
    y = nc.dram_tensor("y", [G_CORE], F32, kind="ExternalOutput")

    with tile.TileContext(nc) as tc:
        with tc.tile_pool(name="consts", bufs=1) as cpool:
            ident_s = cpool.tile([128, 128], F16)
            nc.sync.dma_start(ident_s[:], ident[:])
            ind_s = cpool.tile([128, GP], F16)
            nc.sync.dma_start(ind_s[:], ind[:])
            w0m_s = cpool.tile([128, 256], F32)
            nc.sync.dma_start(w0m_s[:], w0m[:])
            w0x_s = cpool.tile([128, 256], F32)
            nc.sync.dma_start(w0x_s[:], w0x[:])
            w1a_s = cpool.tile([128, 128], F32)
            nc.sync.dma_start(w1a_s[:], w1[0:128, :])
            w1b_s = cpool.tile([128, 128], F32)
            nc.sync.dma_start(w1b_s[:], w1[128:256, :])
            w2_s = cpool.tile([128, 1], F32)
            nc.sync.dma_start(w2_s[:], w2[:])
            b0_s = cpool.tile([128, 2], F32)
            nc.sync.dma_start(b0_s[:], b0[:].rearrange("(h p) -> p h", p=128))
            b1_s = cpool.tile([128, 1], F32)
            nc.sync.dma_start(b1_s[:], b1[:].rearrange("(p o) -> p o", o=1))
            b2_s = cpool.tile([1, 1], F32)
            nc.sync.dma_start(b2_s[:], b2[:].rearrange("(p o) -> p o", o=1))

            pooledT = cpool.tile([128, 2 * G_CORE], F32, tag="pooledT")
            meanT = pooledT[:, 0:G_CORE]
            maxT = pooledT[:, G_CORE : 2 * G_CORE]
            h1 = cpool.tile([128, 2 * G_CORE], F32, tag="h1")
            h2 = cpool.tile([128, G_CORE], F32, tag="h2")
            ysb = cpool.tile([1, G_CORE], F32, tag="ysb")

            # MLP for one block of graphs [g0, g0+gn); issued as soon as the
            # superperiods covering the block have produced meanT/maxT.
            def emit_mlp_block(mlp_pool, g0, gn):
                for h in range(2):
                    pm = mlp_pool.tile([128, 512], F32, tag="pm")
                    nc.tensor.matmul(
                        pm[:, 0:gn],
                        lhsT=w0m_s[:, h * 128 : (h + 1) * 128],
                        rhs=meanT[:, g0 : g0 + gn],
                        start=True,
                        stop=False,
                        skip_group_check=True,
                    )
                    nc.tensor.matmul(
                        pm[:, 0:gn],
                        lhsT=w0x_s[:, h * 128 : (h + 1) * 128],
                        rhs=maxT[:, g0 : g0 + gn],
                        start=False,
                        stop=True,
                        skip_group_check=True,
                    )
                    nc.scalar.activation(
                        h1[:, h * G_CORE + g0 : h * G_CORE + g0 + gn],
                        pm[:, 0:gn],
                        AF.Relu,
                        bias=b0_s[:, h : h + 1],
                    )
                pm = mlp_pool.tile([128, 512], F32, tag="pm")
                nc.tensor.matmul(
                    pm[:, 0:gn],
                    lhsT=w1a_s[:],
                    rhs=h1[:, g0 : g0 + gn],
                    start=True,
                    stop=False,
                    skip_group_check=True,
                )
                nc.tensor.matmul(
                    pm[:, 0:gn],
                    lhsT=w1b_s[:],
                    rhs=h1[:, G_CORE + g0 : G_CORE + g0 + gn],
                    start=False,
                    stop=True,
                    skip_group_check=True,
                )
                nc.scalar.activation(
                    h2[:, g0 : g0 + gn], pm[:, 0:gn], AF.Relu, bias=b1_s[:],
                )
                pm1 = mlp_pool.tile([1, 512], F32, tag="pm1")
                nc.tensor.matmul(
                    pm1[:, 0:gn],
                    lhsT=w2_s[:],
                    rhs=h2[:, g0 : g0 + gn],
                    start=True,
                    stop=True,
                    skip_group_check=True,
                )
                nc.scalar.activation(
                    ysb[:, g0 : g0 + gn], pm1[:, 0:gn], AF.Sigmoid, bias=b2_s[:],
                )
                nc.sync.dma_start(y[g0 : g0 + gn], ysb[:, g0 : g0 + gn])

            def emit_body():
                # graph blocks for the interleaved MLP: emitted after the
                # superperiod that completes each block's pooled columns.
                mlp_after = {7: (0, 512), 15: (512, 512), N_SP - 1: (1024, 256)}
                with (
                    tc.tile_pool(name="chunks", bufs=6) as chunk_pool,
                    tc.tile_pool(name="tree", bufs=2) as tree_pool,
                    tc.tile_pool(name="parts", bufs=2) as part_pool,
                    tc.tile_pool(name="ptr", bufs=2, space="PSUM") as pt_psum,
                    tc.tile_pool(name="pmean", bufs=2, space="PSUM") as mean_psum,
                    tc.tile_pool(name="pmlp", bufs=2, space="PSUM") as mlp_pool,
                ):
                    for sp in range(N_SP):
                        psp = part_pool.tile([128, SP * 128], F16, tag="psp")
                        pT = pt_psum.tile([128, SP * 128], F16, tag="pT")
                        pmean = mean_psum.tile([128, SP * GP], F32, tag="pmean")
                        for k in range(SP):
                            per = sp * SP + k
                            chunk = chunk_pool.tile([128, JCOLS * D], F16, tag="chunk")
                            src = feats[
                                per * PERIOD_NODES : (per + 1) * PERIOD_NODES, :
                            ].rearrange("(p r) d -> p (r d)", p=128)
                            nc.gpsimd.dma_start(chunk[:], src)  # fp32->fp16 cast

                            # fp16 tensor_tensor max tree over the 25 node
                            # slices: 25 -> 12(+r24) -> 6 -> 3 -> 1
                            t1 = tree_pool.tile([128, 12 * D], F16, tag="t1")
                            nc.vector.tensor_tensor(
                                t1[:], chunk[:, 0 : 12 * D],
                                chunk[:, 12 * D : 24 * D], op=ALU.max)
                            t2 = tree_pool.tile([128, 6 * D], F16, tag="t2")
                            nc.vector.tensor_tensor(
                                t2[:], t1[:, 0 : 6 * D],
                                t1[:, 6 * D : 12 * D], op=ALU.max)
                            t3 = tree_pool.tile([128, 3 * D], F16, tag="t3")
                            nc.vector.tensor_tensor(
                                t3[:], t2[:, 0 : 3 * D],
                                t2[:, 3 * D : 6 * D], op=ALU.max)
                            t4 = tree_pool.tile([128, D], F16, tag="t4")
                            nc.vector.tensor_tensor(
                                t4[:], t3[:, 0:D], t3[:, D : 2 * D], op=ALU.max)
                            t5 = tree_pool.tile([128, D], F16, tag="t5")
                            nc.vector.tensor_tensor(
                                t5[:], t4[:], t3[:, 2 * D : 3 * D], op=ALU.max)
                            nc.vector.tensor_tensor(
                                psp[:, k * 128 : (k + 1) * 128],
                                t5[:], chunk[:, 24 * D : 25 * D], op=ALU.max)

                            # exact fp32 per-graph sums via 0/1 indicator
                            for j in range(JCOLS):
                                nc.tensor.matmul(
                                    pmean[:, k * GP : (k + 1) * GP],
                                    lhsT=chunk[:, j * D : (j + 1) * D],
                                    rhs=ind_s[:],
                                    start=(j == 0),
                                    stop=(j == JCOLS - 1),
                                    skip_group_check=True,
                                )
                            # transpose this period's partial into PSUM
                            nc.tensor.matmul(
                                pT[:, k * 128 : (k + 1) * 128],
                                lhsT=psp[:, k * 128 : (k + 1) * 128],
                                rhs=ident_s[:],
                                is_transpose=True,
                                skip_group_check=True,
                            )
                        # per-superperiod: max over the m=8 partitions/graph
                        nc.vector.reduce_max(
                            maxT[:, sp * SP * GP : (sp + 1) * SP * GP],
                            pT[:].rearrange(
                                "d (k g m) -> d (k g) m", k=SP, g=GP, m=8
                            ),
                            axis=AX.X,
                        )
                        nc.scalar.copy(
                            meanT[:, sp * SP * GP : (sp + 1) * SP * GP], pmean[:]
                        )
                        if sp in mlp_after:
                            emit_mlp_block(mlp_pool, *mlp_after[sp])
                if pooled_dbg is not None:
                    nc.sync.dma_start(pooled_dbg[:], pooledT[:])

            if reps == 1:
                emit_body()
            else:
                with tc.For_i(0, reps, 1):
                    emit_body()

    nc.finalize()
    return nc


def _host_constants(W0, b0, W1, b1, W2, b2, scale):
    """Host-side constant prep (fp32/fp16 numpy)."""
    ident = np.eye(128, dtype=np.float16)
    ind = np.zeros((128, GP), dtype=np.float16)
    for p in range(128):
        ind[p, p // 8] = 1.0
    w0m = (np.asarray(W0[0:D, :], dtype=np.float32) * scale).astype(np.float32)
    w0x = np.ascontiguousarray(np.asarray(W0[D : 2 * D, :], dtype=np.float32))
    return {
        "ident": ident,
        "ind": ind,
        "w0m": w0m,
        "w0x": w0x,
        "w1": np.ascontiguousarray(np.asarray(W1, dtype=np.float32)),
        "w2": np.ascontiguousarray(np.asarray(W2, dtype=np.float32)),
        "b0": np.ascontiguousarray(np.asarray(b0, dtype=np.float32)),
        "b1": np.ascontiguousarray(np.asarray(b1, dtype=np.float32)),
        "b2": np.ascontiguousarray(np.asarray(b2, dtype=np.float32)),
    }


_PROGRAM_CACHE: dict = {}


def _get_program(reps: int = 1):
    if reps not in _PROGRAM_CACHE:
        _PROGRAM_CACHE[reps] = build_program(reps)
    return _PROGRAM_CACHE[reps]


def _numpy_fallback(self_feats, graph_size, W0, b0, W1, b1, W2, b2):
    """Pure-numpy reference path for non-uniform graph sizes (never hit with
    the standard setup_inputs, which is uniform 200)."""
    sizes = np.asarray(graph_size, dtype=np.int64)
    G = sizes.shape[0]
    x = np.asarray(self_feats, dtype=np.float32)
    offs = np.concatenate([[0], np.cumsum(sizes)])
    mean_feats = np.empty((G, x.shape[1]), np.float32)
    max_feats = np.empty((G, x.shape[1]), np.float32)
    for g in range(G):
        seg = x[offs[g] : offs[g + 1]]
        mean_feats[g] = seg.mean(axis=0)
        max_feats[g] = seg.max(axis=0)
    pooled = np.concatenate([mean_feats, max_feats], axis=1)
    h = np.maximum(pooled @ np.asarray(W0, np.float32) + np.asarray(b0, np.float32), 0)
    h = np.maximum(h @ np.asarray(W1, np.float32) + np.asarray(b1, np.float32), 0)
    z = h @ np.asarray(W2, np.float32) + np.asarray(b2, np.float32)
    return (1.0 / (1.0 + np.exp(-z))).astype(np.float32)


def kernel(self_feats, graph_size, W0, b0, W1, b1, W2, b2):
    sizes = np.asarray(graph_size)
    x = np.asarray(self_feats, dtype=np.float32)
    if not (
        sizes.shape == (N_GRAPHS,)
        and np.all(sizes == NPG)
        and x.shape == (N_GRAPHS * NPG, D)
    ):
        return _numpy_fallback(self_feats, graph_size, W0, b0, W1, b1, W2, b2)

    consts = _host_constants(W0, b0, W1, b1, W2, b2, 1.0 / NPG)
    in_maps = []
    for c in range(NCORES):
        r0 = CORE_G0[c] * NPG
        m = {"feats": x[r0 : r0 + CORE_ROWS, :]}
        m.update(consts)
        in_maps.append(m)

    nc = _get_program(1)
    res = run_bass_kernel_spmd(nc, in_maps, list(range(NCORES)))

    out = np.empty((N_GRAPHS, 1), dtype=np.float32)
    for c in range(NCORES):
        keep0 = 0 if c < 7 else (1250 * 7 - CORE_G0[7])
        yc = res.results[c]["y"]
        out[c * PER_CORE_OUT : (c + 1) * PER_CORE_OUT, 0] = yc[
            keep0 : keep0 + PER_CORE_OUT
        ]
    return out
